# revision 1
# baseline (speedup 1.0000x reference)
"""AutoMTLSuperNet (moe_routing) Trainium2 kernel.

Strategy: batch data-parallel over 8 NeuronCores (2048 samples each, params
replicated). On-chip layout is output-channel-major ([oc, batch]) so every
layer's matmul output is directly the next layer's moving operand. All
matmuls run in bf16 with f32 PSUM accumulation; batch is processed in
chunks of 512 columns.

Host-side prep is parameter-only + input layout: transposes, padding,
folding sigmoid(feat_alpha) into the L0 weight rows, FM contraction
matrices, candidate-softmax weights, merged gate biases.
"""

import numpy as np
import ml_dtypes

import concourse.bass as bass
import concourse.bacc as bacc
import concourse.mybir as mybir
import concourse.tile as tile
from concourse.bass_utils import run_bass_kernel_spmd

# ---- problem dims (hardcoded per contract) ----
B, F, E, D = 16384, 26, 16, 13
NE, ND, NC = 4, 3, 3
GIN = E * (F + 1) + D            # 445
H, OUT = 256, 128
N_CORES = 8
B_LOC = B // N_CORES             # 2048
NBC = 512                        # batch columns per chunk
NCHUNK = B_LOC // NBC            # 4
KSP = F * E                      # 416 flattened sparse dim
KPAD = 448                       # padded to 4 x (128,128,128,64)
BF16 = mybir.dt.bfloat16
F32 = mybir.dt.float32

AF = mybir.ActivationFunctionType
ALU = mybir.AluOpType


def _bf16(x):
    return np.asarray(x, dtype=ml_dtypes.bfloat16)


def _softmax_np(a):
    a = np.asarray(a, dtype=np.float64)
    m = a.max(axis=-1, keepdims=True)
    e = np.exp(a - m)
    return (e / e.sum(axis=-1, keepdims=True)).astype(np.float32)


def prep_shared(inputs):
    """Host prep of all parameter tensors (input-layout + parameter-only math)."""
    f32 = np.float32
    gate_w = 1.0 / (1.0 + np.exp(-inputs['feat_alpha'].astype(np.float64)))  # [NE,F]
    gate_w = gate_w.astype(f32)

    W_l0b0 = inputs['W_l0b0'].astype(f32)   # [NE,NC,GIN,H]
    W_l0b1 = inputs['W_l0b1'].astype(f32)   # [NE,NC,H,OUT]
    W_l1b0 = inputs['W_l1b0'].astype(f32)   # [NE,NC,OUT,H]
    W_l1b1 = inputs['W_l1b1'].astype(f32)   # [NE,NC,H,OUT]

    # candidate softmax weights per mixed-op layer: [4][NE,NC]
    wmix_l = [_softmax_np(inputs[k]) for k in ('a_l0b0', 'a_l0b1', 'a_l1b0', 'a_l1b1')]

    # ---- Wl0: lhsT ktiles [4,128, 3072]; col = n*768 + c*256 + h ----
    Wl0 = np.zeros((4, 128, NE * NC * H), dtype=f32)
    # sparse rows (g-folded): global row i = fe for fe in 0..415
    Wsp = np.zeros((KSP, NE, NC, H), dtype=f32)
    for n in range(NE):
        gvec = np.repeat(gate_w[n], E)                      # [416]
        Wsp[:, n] = W_l0b0[n, :, :KSP, :].transpose(1, 0, 2) * gvec[:, None, None]
    Wsp = Wsp.reshape(KSP, NE * NC * H)
    for kt in range(3):
        Wl0[kt, :, :] = Wsp[kt * 128:(kt + 1) * 128]
    # kt3 layout: [0:32]=sparse rows 384..415, [32:45]=dense, [45:64]=0,
    #             [64:128]=fm rows (64 + n*16 + e)
    Wl0[3, 0:32, :] = Wsp[384:416]
    for d in range(D):
        Wl0[3, 32 + d, :] = W_l0b0[:, :, KSP + E + d, :].reshape(-1)
    for n in range(NE):
        for e in range(E):
            Wl0[3, 64 + n * 16 + e, n * 768:(n + 1) * 768] = \
                W_l0b0[n, :, KSP + e, :].reshape(768)

    # ---- Gs / Gq: [4,128,64]  col = n*16+e ; row = fe (per ktile of xT) ----
    Gs = np.zeros((4, 128, 64), dtype=f32)
    Gq = np.zeros((4, 128, 64), dtype=f32)
    for fe in range(KSP):
        kt, i = divmod(fe, 128)
        f_, e_ = divmod(fe, E)
        for n in range(NE):
            g = gate_w[n, f_]
            Gs[kt, i, n * 16 + e_] = g
            Gq[kt, i, n * 16 + e_] = 0.5 * g * g   # 0.5 pre-folded
    # ---- Wg: [4,128,44]: cols 0-15 g0 (e*4+n), 32-43 g1 (32+d*4+e) ----
    Wg = np.zeros((4, 128, 44), dtype=f32)
    Wg0, Wg1 = inputs['Wg0'].astype(f32), inputs['Wg1'].astype(f32)
    for i in range(KSP):
        kt, r = divmod(i, 128)
        for n in range(NE):
            for e in range(NE):
                Wg[kt, r, e * 4 + n] = Wg0[n, i, e]
        for d in range(ND):
            for e in range(NE):
                Wg[kt, r, 32 + d * 4 + e] = Wg1[d, i, e]
    gbias = np.zeros((44, 1), dtype=f32)
    for n in range(NE):
        for e in range(NE):
            gbias[e * 4 + n, 0] = inputs['bg0'][n, e] + inputs['beta0'][n, e]
    for d in range(ND):
        for e in range(NE):
            gbias[32 + d * 4 + e, 0] = inputs['bg1'][d, e] + inputs['beta1'][d, e]
    # sel16 [16,4]: row e*4+n -> col n
    sel16 = np.zeros((16, 4), dtype=f32)
    for e in range(NE):
        for n in range(NE):
            sel16[e * 4 + n, n] = 1.0

    # ---- later layer weights ----
    Wb1 = np.zeros((NE, H, NC * OUT), dtype=f32)       # lhsT col = c*128+o
    for n in range(NE):
        Wb1[n] = W_l0b1[n].transpose(1, 0, 2).reshape(H, NC * OUT)
    W10 = np.zeros((NE, OUT, NC * H), dtype=f32)       # col = c*256+h
    for n in range(NE):
        W10[n] = W_l1b0[n].transpose(1, 0, 2).reshape(OUT, NC * H)
    W11 = np.zeros((NE, H, NC * OUT), dtype=f32)
    for n in range(NE):
        W11[n] = W_l1b1[n].transpose(1, 0, 2).reshape(H, NC * OUT)

    # ---- bias column tables (per-partition vectors), w-scaled for relu c=0 ----
    def bias_table(bmat, wl, n_mt):  # bmat [NE,NC,W]; returns [128, NE*NC*n_mt]
        Wd = bmat.shape[-1]
        tbl = np.zeros((128, NE * NC * (Wd // 128)), dtype=f32)
        m = 0
        for n in range(NE):
            for c in range(NC):
                for hh in range(Wd // 128):
                    v = bmat[n, c, hh * 128:(hh + 1) * 128].astype(f32)
                    if c == 0:
                        v = v * wl[n, 0]
                    tbl[:, m] = v
                    m += 1
        return tbl
    bl0b0 = bias_table(inputs['b_l0b0'], wmix_l[0], 2)   # [128,24]
    bl0b1 = bias_table(inputs['b_l0b1'], wmix_l[1], 1)   # [128,12]
    bl1b0 = bias_table(inputs['b_l1b0'], wmix_l[2], 2)   # [128,24]
    bl1b1 = bias_table(inputs['b_l1b1'], wmix_l[3], 1)   # [128,12]

    wmix = np.zeros((128, 48), dtype=f32)
    for li, wl in enumerate(wmix_l):
        for n in range(NE):
            for c in range(NC):
                wmix[:, li * 12 + n * 3 + c] = wl[n, c]

    ident = np.eye(128, dtype=f32)
    ones1 = np.ones((1, 128), dtype=f32)
    # broadcast selectors: selbc[r] = e_r (x) ones128  -> lhsT picks row r of rhs
    selbc = np.zeros((16, 16, 128), dtype=f32)
    for r_ in range(16):
        selbc[r_, r_, :] = 1.0
    selbr = np.zeros((4, 4, 128), dtype=f32)
    for r_ in range(4):
        selbr[r_, r_, :] = 1.0

    shared = {
        'Wl0': _bf16(Wl0), 'Gs': _bf16(Gs), 'Gq': _bf16(Gq), 'Wg': _bf16(Wg),
        'sel16': _bf16(sel16), 'Wb1': _bf16(Wb1), 'W10': _bf16(W10),
        'W11': _bf16(W11), 'gbias': gbias,
        'bl0b0': bl0b0, 'bl0b1': bl0b1, 'bl1b0': bl1b0, 'bl1b1': bl1b1,
        'wmix': wmix, 'ident': _bf16(ident),
        'selbc': _bf16(selbc), 'selbr': _bf16(selbr),
    }
    return shared


def prep_core(inputs, r):
    """Per-core input shards (layout only)."""
    lo, hi = r * B_LOC, (r + 1) * B_LOC
    xs = inputs['sparse_embs'][lo:hi].reshape(B_LOC, KSP)      # [2048,416] f32
    xT = np.zeros((KPAD, B_LOC), dtype=ml_dtypes.bfloat16)
    xT[:KSP] = _bf16(xs.T)
    dxT = _bf16(inputs['dense_features'][lo:hi].astype(np.float32).T)  # [13,2048]
    domc = inputs['domain_ids'][lo:hi].astype(np.float32).reshape(B_LOC, 1)
    return {'xT': xT, 'dxT': dxT, 'domc': domc}


def build_program(relu_dve=True):
    """relu_dve: move L0b0/L1b0 relu branches to DVE tensor_scalar (max,mult).
    Only valid when b_l0b0/b_l1b0 are all-zero (checked by caller)."""
    nc = bacc.Bacc(trn_type="TRN2", target_bir_lowering=False, debug=False)

    # ---- DRAM I/O ----
    t_xT = nc.dram_tensor('xT', [KPAD, B_LOC], BF16, kind="ExternalInput").ap()
    t_dxT = nc.dram_tensor('dxT', [D, B_LOC], BF16, kind="ExternalInput").ap()
    t_domc = nc.dram_tensor('domc', [B_LOC, 1], F32, kind="ExternalInput").ap()
    t_Wl0 = nc.dram_tensor('Wl0', [4, 128, 3072], BF16, kind="ExternalInput").ap()
    t_Gs = nc.dram_tensor('Gs', [4, 128, 64], BF16, kind="ExternalInput").ap()
    t_Gq = nc.dram_tensor('Gq', [4, 128, 64], BF16, kind="ExternalInput").ap()
    t_Wg = nc.dram_tensor('Wg', [4, 128, 44], BF16, kind="ExternalInput").ap()
    t_sel16 = nc.dram_tensor('sel16', [16, 4], BF16, kind="ExternalInput").ap()
    t_Wb1 = nc.dram_tensor('Wb1', [NE, H, 384], BF16, kind="ExternalInput").ap()
    t_W10 = nc.dram_tensor('W10', [NE, OUT, 768], BF16, kind="ExternalInput").ap()
    t_W11 = nc.dram_tensor('W11', [NE, H, 384], BF16, kind="ExternalInput").ap()
    t_gbias = nc.dram_tensor('gbias', [44, 1], F32, kind="ExternalInput").ap()
    t_bl0b0 = nc.dram_tensor('bl0b0', [128, 24], F32, kind="ExternalInput").ap()
    t_bl0b1 = nc.dram_tensor('bl0b1', [128, 12], F32, kind="ExternalInput").ap()
    t_bl1b0 = nc.dram_tensor('bl1b0', [128, 24], F32, kind="ExternalInput").ap()
    t_bl1b1 = nc.dram_tensor('bl1b1', [128, 12], F32, kind="ExternalInput").ap()
    t_wmix = nc.dram_tensor('wmix', [128, 48], F32, kind="ExternalInput").ap()
    t_ident = nc.dram_tensor('ident', [128, 128], BF16, kind="ExternalInput").ap()
    t_selbc = nc.dram_tensor('selbc', [16, 16, 128], BF16, kind="ExternalInput").ap()
    t_selbr = nc.dram_tensor('selbr', [4, 4, 128], BF16, kind="ExternalInput").ap()
    t_out = nc.dram_tensor('out', [B_LOC, OUT], F32, kind="ExternalOutput").ap()

    KT_ROWS = [128, 128, 128, 64]   # xT sbuf k-tiling
    K3 = 128

    with tile.TileContext(nc) as tc:
        with (
            tc.tile_pool(name="wpool", bufs=1) as wpool,
            tc.tile_pool(name="xpool", bufs=4) as xpool,
            tc.tile_pool(name="apool", bufs=2) as apool,
            tc.tile_pool(name="hpool", bufs=2) as hpool,
            tc.tile_pool(name="bcpool", bufs=4) as bcpool,
            tc.tile_pool(name="spool", bufs=4) as spool,
            tc.tile_pool(name="opool", bufs=2) as opool,
            tc.tile_pool(name="ps_mm", bufs=3, space="PSUM") as ps_mm,
            tc.tile_pool(name="ps_smlt", bufs=2, space="PSUM") as ps_smlt,
            tc.tile_pool(name="ps_bc", bufs=3, space="PSUM") as ps_bc,
        ):
            # ---- prologue: resident weights/constants ----
            def wtile(src_ap, shape, dtype=BF16, tag=None):
                t = wpool.tile(shape, dtype, tag=tag, name=tag)
                nc.sync.dma_start(t[:], src_ap)
                return t

            sWl0 = [wtile(t_Wl0[kt], [128, 3072], tag=f"Wl0_{kt}") for kt in range(4)]
            sGs = [wtile(t_Gs[kt][:KT_ROWS[kt]], [KT_ROWS[kt], 64], tag=f"Gs{kt}") for kt in range(4)]
            sGq = [wtile(t_Gq[kt][:KT_ROWS[kt]], [KT_ROWS[kt], 64], tag=f"Gq{kt}") for kt in range(4)]
            sWg = [wtile(t_Wg[kt][:KT_ROWS[kt]], [KT_ROWS[kt], 44], tag=f"Wg{kt}") for kt in range(4)]
            sSel = wtile(t_sel16, [16, 4], tag="sel16")
            sWb1 = [[wtile(t_Wb1[n][kt * 128:(kt + 1) * 128, :], [128, 384],
                           tag=f"Wb1_{n}{kt}") for kt in range(2)] for n in range(NE)]
            sW10 = [wtile(t_W10[n], [OUT, 768], tag=f"W10_{n}") for n in range(NE)]
            sW11 = [[wtile(t_W11[n][kt * 128:(kt + 1) * 128, :], [128, 384],
                           tag=f"W11_{n}{kt}") for kt in range(2)] for n in range(NE)]
            sGb = wtile(t_gbias, [44, 1], F32, tag="gbias")
            sB00 = wtile(t_bl0b0, [128, 24], F32, tag="bl0b0")
            sB01 = wtile(t_bl0b1, [128, 12], F32, tag="bl0b1")
            sB10 = wtile(t_bl1b0, [128, 24], F32, tag="bl1b0")
            sB11 = wtile(t_bl1b1, [128, 12], F32, tag="bl1b1")
            sWmix = wtile(t_wmix, [128, 48], F32, tag="wmix")
            sId = wtile(t_ident, [128, 128], tag="ident")
            sSelBc = [wtile(t_selbc[r], [16, 128], tag=f"selbc{r}") for r in range(16)]
            sSelBr = [wtile(t_selbr[r], [4, 128], tag=f"selbr{r}") for r in range(4)]

            # per-chunk state carried between phases
            xk = [None] * NCHUNK
            hyb = [None] * NCHUNK
            domt = [None] * NCHUNK
            e0bf = [None] * NCHUNK
            e1bf = [None] * NCHUNK
            r0 = [None] * NCHUNK
            hA = [None] * NCHUNK
            hB = [None] * NCHUNK
            mixed = [None] * NCHUNK
            hC = [None] * NCHUNK
            h2 = [None] * NCHUNK

            def mixed_op_tail(p, out_t, c, bcol, wcol, relu_on_dve, tmp_tag):
                """candidate-mix tail for one [128,NBC] branch psum tile."""
                if c == 0:
                    if relu_on_dve:
                        nc.vector.tensor_scalar(out_t[:], p[:], 0.0, wcol,
                                                ALU.max, ALU.mult)
                    else:
                        nc.scalar.activation(out_t[:], p[:], AF.Relu,
                                             bias=bcol, scale=wcol)
                else:
                    fn = AF.Gelu_apprx_tanh if c == 1 else AF.Tanh
                    tmp = apool.tile([128, NBC], BF16, tag=tmp_tag,
                                     name=f"t{tmp_tag}_{next(uid)}")
                    nc.scalar.activation(tmp[:], p[:], fn, bias=bcol)
                    tw = apool.tile([128, NBC], BF16, tag="tw" + tmp_tag,
                                    name=f"w{tmp_tag}_{next(uid)}")
                    nc.vector.tensor_scalar(tw[:], tmp[:], wcol, None, ALU.mult)
                    nc.vector.tensor_tensor(out_t[:], out_t[:], tw[:], ALU.add)

            import itertools
            uid = itertools.count()

            # ============ P0: loads, squares, fm, gates, softmax prep ============
            def phase0(ch):
                cc = ch * NBC
                xk[ch] = []
                for kt in range(4):
                    t = xpool.tile([KT_ROWS[kt], NBC], BF16, tag=f"x{kt}", name=f"x{kt}_{ch}")
                    nc.sync.dma_start(t[:], t_xT[kt * 128: kt * 128 + KT_ROWS[kt], cc:cc + NBC])
                    xk[ch].append(t)
                dx = xpool.tile([D, NBC], BF16, tag="dx", name=f"dx_{ch}")
                nc.sync.dma_start(dx[:], t_dxT[:, cc:cc + NBC])
                domt[ch] = []
                for bt in range(4):
                    t = xpool.tile([128, 1], F32, tag=f"dom{bt}", name=f"dom{bt}_{ch}")
                    nc.sync.dma_start(t[:], t_domc[cc + bt * 128: cc + (bt + 1) * 128, :])
                    domt[ch].append(t)

                xq = []
                for kt in range(4):
                    t = xpool.tile([KT_ROWS[kt], NBC], BF16, tag=f"xq{kt}", name=f"xq{kt}_{ch}", bufs=2)
                    nc.vector.tensor_tensor(t[:], xk[ch][kt][:], xk[ch][kt][:], ALU.mult)
                    xq.append(t)

                sq_ps = ps_smlt.tile([128, NBC], F32, tag="smlt", name=f"sq_{ch}")
                for kt in range(4):
                    nc.tensor.matmul(sq_ps[0:64, :], sGs[kt][:], xk[ch][kt][:],
                                     start=(kt == 0), stop=(kt == 3))
                for kt in range(4):
                    nc.tensor.matmul(sq_ps[64:128, :], sGq[kt][:], xq[kt][:],
                                     start=(kt == 0), stop=(kt == 3))
                ssq = spool.tile([64, NBC], F32, tag="ssq", name=f"ssq_{ch}")
                nc.scalar.activation(ssq[:], sq_ps[0:64, :], AF.Square,
                                     scale=float(np.sqrt(0.5)))
                hyb[ch] = xpool.tile([K3, NBC], BF16, tag="hyb", name=f"hyb_{ch}")
                nc.vector.memset(hyb[ch][32:64, :], 0.0)
                nc.vector.tensor_copy(hyb[ch][0:32, :], xk[ch][3][0:32, :])
                nc.vector.tensor_copy(hyb[ch][32:45, :], dx[:])
                nc.vector.tensor_tensor(hyb[ch][64:128, :], ssq[:], sq_ps[64:128, :],
                                        ALU.subtract)

                g_ps = ps_smlt.tile([44, NBC], F32, tag="smlt", name=f"g_{ch}")
                for kt in range(4):
                    nc.tensor.matmul(g_ps[:], sWg[kt][:], xk[ch][kt][:],
                                     start=(kt == 0), stop=(kt == 3))
                gexp = spool.tile([44, NBC], F32, tag="gexp", name=f"gexp_{ch}")
                nc.scalar.activation(gexp[:], g_ps[:], AF.Exp, bias=sGb[:, 0:1])
                e0 = spool.tile([16, NBC], BF16, tag="e0bf", name=f"e0_{ch}")
                nc.vector.tensor_copy(e0[:], gexp[0:16, :])
                e0bf[ch] = e0
                e1 = spool.tile([12, NBC], BF16, tag="e1bf", name=f"e1_{ch}")
                nc.vector.tensor_copy(e1[:], gexp[32:44, :])
                e1bf[ch] = e1
                s_ps = ps_smlt.tile([4, NBC], F32, tag="smlt", name=f"s0_{ch}")
                nc.tensor.matmul(s_ps[:], sSel[:], e0[:], start=True, stop=True)
                r = spool.tile([4, NBC], BF16, tag="r0", name=f"r0_{ch}")
                with nc.allow_low_precision("softmax recip feeds bf16 mix"):
                    nc.vector.reciprocal(r[:], s_ps[:])
                r0[ch] = r

            # ============ P1: L0b0 + mixA -> hA ; L0b1 + mixB -> hB ============
            def phase1(ch):
                hA[ch] = {}
                for n in range(NE):
                    for hh in range(2):
                        hA[ch][(n, hh)] = hpool.tile([128, NBC], BF16, tag=f"hA{n}{hh}",
                                                     name=f"hA{n}{hh}_{ch}")
                    for c in range(NC):
                        for hh in range(2):
                            m = n * 6 + c * 2 + hh
                            p = ps_mm.tile([128, NBC], F32, tag="pmm", name=f"pA{m}_{ch}")
                            for kt in range(3):
                                nc.tensor.matmul(p[:], sWl0[kt][:, m * 128:(m + 1) * 128],
                                                 xk[ch][kt][:], start=(kt == 0), stop=False)
                            nc.tensor.matmul(p[:], sWl0[3][0:K3, m * 128:(m + 1) * 128],
                                             hyb[ch][:], start=False, stop=True)
                            mixed_op_tail(p, hA[ch][(n, hh)], c, sB00[:, m:m + 1],
                                          sWmix[:, n * 3 + c: n * 3 + c + 1],
                                          relu_dve, f"A{hh}")
                hB[ch] = {}
                for n in range(NE):
                    hb = hpool.tile([128, NBC], BF16, tag=f"hB{n}", name=f"hB{n}_{ch}")
                    hB[ch][n] = hb
                    for c in range(NC):
                        p = ps_mm.tile([128, NBC], F32, tag="pmm", name=f"pB{n}{c}_{ch}")
                        for kt in range(2):
                            nc.tensor.matmul(p[:], sWb1[n][kt][:, c * 128:(c + 1) * 128],
                                             hA[ch][(n, kt)][:], start=(kt == 0), stop=(kt == 1))
                        m = n * 3 + c
                        mixed_op_tail(p, hb, c, sB01[:, m:m + 1],
                                      sWmix[:, 12 + m: 12 + m + 1], False, "B")

            # ============ P2: expert mixing 0 ============
            def phase2(ch):
                mixed[ch] = {}
                for n in range(NE):
                    acc = None
                    bcb = []
                    for e in range(NE):
                        bp = ps_bc.tile([128, NBC], F32, tag="bcp", name=f"bcp{n}{e}_{ch}")
                        nc.tensor.matmul(bp[:], sSelBc[e * 4 + n][:], e0bf[ch][:],
                                         start=True, stop=True)
                        # ACT copies psum->sbuf bf16; DVE multiplies from sbuf
                        bb = bcpool.tile([128, NBC], BF16, tag="bcb", name=f"bcb{n}{e}_{ch}")
                        nc.scalar.copy(bb[:], bp[:])
                        bcb.append(bb)
                    rp = ps_bc.tile([128, NBC], F32, tag="bcp", name=f"rp{n}_{ch}")
                    nc.tensor.matmul(rp[:], sSelBr[n][:], r0[ch][:], start=True, stop=True)
                    acc = bcpool.tile([128, NBC], BF16, tag="mixacc", name=f"acc{n}_{ch}")
                    nc.vector.tensor_tensor(acc[:], hB[ch][0][:], bcb[0][:], ALU.mult)
                    for e in range(1, NE):
                        t2 = bcpool.tile([128, NBC], BF16, tag="mixt", name=f"mixt{n}{e}_{ch}")
                        nc.vector.tensor_tensor(t2[:], hB[ch][e][:], bcb[e][:], ALU.mult)
                        nc.vector.tensor_tensor(acc[:], acc[:], t2[:], ALU.add)
                    mx = hpool.tile([128, NBC], BF16, tag=f"mix{n}", name=f"mix{n}_{ch}")
                    nc.vector.tensor_tensor(mx[:], acc[:], rp[:], ALU.mult)
                    mixed[ch][n] = mx

            # ============ P3: L1b0 + mixC -> hC ; L1b1 + mixD -> h2 ============
            def phase3(ch):
                hC[ch] = {}
                for n in range(NE):
                    for hh in range(2):
                        hC[ch][(n, hh)] = hpool.tile([128, NBC], BF16, tag=f"hC{n}{hh}",
                                                     name=f"hC{n}{hh}_{ch}")
                    for c in range(NC):
                        for hh in range(2):
                            m = n * 6 + c * 2 + hh
                            mt = c * 2 + hh
                            p = ps_mm.tile([128, NBC], F32, tag="pmm", name=f"pC{m}_{ch}")
                            nc.tensor.matmul(p[:], sW10[n][:, mt * 128:(mt + 1) * 128],
                                             mixed[ch][n][:], start=True, stop=True)
                            mixed_op_tail(p, hC[ch][(n, hh)], c, sB10[:, m:m + 1],
                                          sWmix[:, 24 + n * 3 + c: 24 + n * 3 + c + 1],
                                          relu_dve, f"C{hh}")
                h2[ch] = {}
                for n in range(NE):
                    hb = hpool.tile([128, NBC], BF16, tag=f"h2{n}", name=f"h2{n}_{ch}")
                    h2[ch][n] = hb
                    for c in range(NC):
                        p = ps_mm.tile([128, NBC], F32, tag="pmm", name=f"pD{n}{c}_{ch}")
                        for kt in range(2):
                            nc.tensor.matmul(p[:], sW11[n][kt][:, c * 128:(c + 1) * 128],
                                             hC[ch][(n, kt)][:], start=(kt == 0), stop=(kt == 1))
                        m = n * 3 + c
                        mixed_op_tail(p, hb, c, sB11[:, m:m + 1],
                                      sWmix[:, 36 + m: 36 + m + 1], False, "Dx")

            # ============ P4: domain softmax-select (batch-major) + out ============
            def phase4(ch):
                cc = ch * NBC
                for bt in range(4):
                    bs = bt * 128
                    tpe = ps_smlt.tile([128, 128], BF16, tag="smlt", name=f"tpe{bt}_{ch}")
                    nc.tensor.transpose(tpe[:, 0:12], e1bf[ch][:, bs:bs + 128],
                                        sId[0:12, 0:12])
                    wsel = opool.tile([128, 4], F32, tag="wsel", name=f"wsel{bt}_{ch}")
                    for d in range(ND):
                        msk = opool.tile([128, 1], F32, tag=f"msk{d}", name=f"msk{d}{bt}_{ch}")
                        nc.vector.tensor_scalar(msk[:], domt[ch][bt][:], float(d), None,
                                                ALU.is_equal)
                        if d == 0:
                            nc.vector.tensor_scalar(wsel[:], tpe[:, 0:4],
                                                    msk[:, 0:1], None, ALU.mult)
                        else:
                            nc.vector.scalar_tensor_tensor(
                                wsel[:], tpe[:, 4 * d:4 * d + 4], msk[:, 0:1],
                                wsel[:], ALU.mult, ALU.add)
                    sume = opool.tile([128, 1], F32, tag="sume", name=f"sume{bt}_{ch}")
                    nc.vector.tensor_reduce(sume[:], wsel[:], mybir.AxisListType.X,
                                            ALU.add)
                    rs = opool.tile([128, 1], F32, tag="rs", name=f"rs{bt}_{ch}")
                    nc.vector.reciprocal(rs[:], sume[:])
                    acc = opool.tile([128, 128], F32, tag="oacc", name=f"oacc{bt}_{ch}")
                    for e in range(NE):
                        tp = ps_smlt.tile([128, 128], BF16, tag="smlt", name=f"tp{e}{bt}_{ch}")
                        nc.tensor.transpose(tp[:], h2[ch][e][:, bs:bs + 128], sId[:])
                        if e == 0:
                            nc.vector.tensor_scalar(acc[:], tp[:], wsel[:, 0:1], None,
                                                    ALU.mult)
                        else:
                            nc.vector.scalar_tensor_tensor(
                                acc[:], tp[:], wsel[:, e:e + 1], acc[:],
                                ALU.mult, ALU.add)
                    ot = opool.tile([128, 128], F32, tag="otile", name=f"ot{bt}_{ch}")
                    nc.vector.tensor_scalar(ot[:], acc[:], rs[:, 0:1], None, ALU.mult)
                    nc.sync.dma_start(t_out[cc + bs: cc + bs + 128, :], ot[:])

            # ---- emission schedule: P0 all, then rounds with one-chunk lag ----
            for ch in range(NCHUNK):
                phase0(ch)
            for ch in range(NCHUNK):
                phase1(ch)
                if ch > 0:
                    phase2(ch - 1)
                    phase3(ch - 1)
                    phase4(ch - 1)
            phase2(NCHUNK - 1)
            phase3(NCHUNK - 1)
            phase4(NCHUNK - 1)
    nc.compile()
    return nc


_CACHE = {}


def kernel(**inputs):
    shared = prep_shared(inputs)
    in_maps = []
    for r in range(N_CORES):
        m = dict(shared)
        m.update(prep_core(inputs, r))
        in_maps.append(m)
    relu_dve = (np.abs(inputs['b_l0b0']).max() == 0.0
                and np.abs(inputs['b_l1b0']).max() == 0.0)
    key = ('nc', bool(relu_dve))
    if key not in _CACHE:
        _CACHE[key] = build_program(relu_dve=relu_dve)
        _CACHE['nc'] = _CACHE[key]
    nc = _CACHE[key]
    res = run_bass_kernel_spmd(nc, in_maps, core_ids=list(range(N_CORES)))
    out = np.concatenate([res.results[r]['out'] for r in range(N_CORES)], axis=0)
    return out.astype(np.float32)

